# revision 1
# baseline (speedup 1.0000x reference)
"""HGT graph update kernel for 8 Trainium2 NeuronCores.

Sharding: edge-parallel by destination-node range. Core c owns dst nodes
[c*12500, (c+1)*12500); its edges (from both edge sets) are routed to it by
the host. Node tensors are rotated per core so every core runs the same SPMD
program with its own range first. No collectives: per-core outputs are
disjoint row ranges, concatenated on the host.

Device pipeline per core:
  P0: project kt/mt (attention/message weights folded with Wk/Wm on host)
      for all N nodes and q for the own range, via TensorE from a
      DMA-transposed fp16 copy of x.
  P1: per 128-edge group: indirect-gather ktmt[src] and q[dst], score =
      sum_c kt*q per head, w = exp(score) (no max-subtraction needed: scores
      are O(0.1), softmax is shift-invariant), payload = [w*mt | w] and
      indirect scatter-add into a [12544, 72] accumulator. Host pre-groups
      edges into occurrence levels so no dst repeats within a group or in
      adjacent groups (scatter-add races otherwise).
  P2: pooled = numer/denom, gelu, @Wa, weighted skip, layernorm.
"""

import numpy as np

N = 100_000
D = 64
H, C = 8, 8
EPS = 1e-3
RSQRT_C = np.float32(1.0 / np.sqrt(C))
NCORES = 8
NOWN = 12500          # dst nodes per core
NOWNP = 12544         # padded (98*128); rows 12500+ are junk
NJUNK0 = 12500
NPAD = 100352         # 49*2048, x rows padded
GROUP = 128           # edges per indirect DMA
TILE_G = 32           # groups per edge tile (4096 edges)


def _block_diag(W):  # [H, C, C] -> [D, D]
    out = np.zeros((D, D), np.float32)
    for h in range(H):
        out[h * C:(h + 1) * C, h * C:(h + 1) * C] = W[h]
    return out


def _prep_core_edges(src, dst, base):
    """Edges with dst in [base, base+12500): returns (src_rot, dst_loc)
    grouped into occurrence levels; each level padded to GROUP multiple and
    followed by one junk group, so no dst repeats within a group or within
    adjacent groups."""
    sel = (dst >= base) & (dst < base + NOWN)
    s = ((src[sel].astype(np.int64) - base) % N).astype(np.int32)
    d = (dst[sel] - base).astype(np.int32)
    order = np.argsort(d, kind="stable")
    s, d = s[order], d[order]
    uniq, first, counts = np.unique(d, return_index=True, return_counts=True)
    occ = np.arange(d.size) - np.repeat(first, counts)
    lvl_order = np.argsort(occ, kind="stable")
    s, d, occ = s[lvl_order], d[lvl_order], occ[lvl_order]
    out_s, out_d = [], []
    junk = lambda n, k: (NJUNK0 + (np.arange(n) + k) % (NOWNP - NJUNK0)).astype(np.int32)
    for b in range(occ.max() + 1 if occ.size else 0):
        m = occ == b
        ls, ld = s[m], d[m]
        pad = (-ls.size) % GROUP
        out_s.append(ls); out_d.append(ld)
        out_s.append(np.zeros(pad + GROUP, np.int32))
        out_d.append(junk(pad + GROUP, b))
    return np.concatenate(out_s), np.concatenate(out_d)


def _build_and_run(inputs):
    import concourse.bass as bass
    import concourse.tile as tile
    import concourse.mybir as mybir
    from concourse.bass_utils import run_bass_kernel_spmd

    x = np.asarray(inputs["x"], np.float32)
    Wk, bk = np.asarray(inputs["Wk"]), np.asarray(inputs["bk"])
    Wm, bm = np.asarray(inputs["Wm"]), np.asarray(inputs["bm"])
    Wq, bq = np.asarray(inputs["Wq"]), np.asarray(inputs["bq"])
    Wa, ba = np.asarray(inputs["Wa"]), np.asarray(inputs["ba"])
    sc = float(1.0 / (1.0 + np.exp(-np.asarray(inputs["skip_w"])[0])))
    gamma, beta = np.asarray(inputs["ln_gamma"]), np.asarray(inputs["ln_beta"])

    # fold per-set head projections + prior*rsqrtC into the dense weights
    Wcols, bcols = [], []
    for s in (0, 1):
        BDa = _block_diag(np.asarray(inputs[f"Watt{s}"]))
        BDa *= np.repeat(np.asarray(inputs[f"prior{s}"]) * RSQRT_C, C)[None, :]
        BDm = _block_diag(np.asarray(inputs[f"Wmsg{s}"]))
        Wcols += [Wk @ BDa, Wm @ BDm]
        bcols += [bk @ BDa, bm @ BDm]
    Wcols.append(Wq); bcols.append(bq)
    Waug = np.concatenate([np.concatenate(Wcols, 1),
                           np.concatenate(bcols)[None, :]], 0).astype(np.float16)

    # per-core edge arrays
    per_core = []
    maxg = [0, 0]
    for c in range(NCORES):
        base = c * NOWN
        e = []
        for s in (0, 1):
            es, ed = _prep_core_edges(np.asarray(inputs[f"src{s}"]),
                                      np.asarray(inputs[f"dst{s}"]), base)
            e.append((es, ed))
            maxg[s] = max(maxg[s], es.size // GROUP)
        per_core.append(e)
    # pad each set's group count to TILE_G multiple, same on all cores
    ng = [-(-m // TILE_G) * TILE_G for m in maxg]
    NT = ng[0] + ng[1]
    in_maps = []
    for c in range(NCORES):
        base = c * NOWN
        si = np.zeros((NT * GROUP,), np.int32)
        di = np.tile(NJUNK0 + np.arange(GROUP) % (NOWNP - NJUNK0),
                     NT).astype(np.int32)
        off = 0
        for s in (0, 1):
            es, ed = per_core[c][s]
            si[off:off + es.size] = es
            di[off:off + ed.size] = ed
            off = ng[0] * GROUP
        xr = np.roll(x, -base, axis=0)
        x16 = np.zeros((NPAD, D), np.float16)
        x16[:N] = xr.astype(np.float16)
        in_maps.append({
            "x16": x16,
            "xown": np.ascontiguousarray(xr[:NOWNP]),
            "waug": Waug,
            "wa": np.ascontiguousarray(Wa.astype(np.float32)),
            "gb": np.stack([gamma, beta]).astype(np.float32),
            "srcidx": np.ascontiguousarray(si.reshape(NT, GROUP).T),
            "dstidx": np.ascontiguousarray(di.reshape(NT, GROUP).T),
        })

    _APPLY_GB = not (np.allclose(gamma, 1.0) and np.allclose(beta, 0.0))
    nc = bass.Bass()
    dt = mybir.dt
    x16_p = nc.declare_dram_parameter("x16", [NPAD, D], dt.float16, isOutput=False)
    xown_p = nc.declare_dram_parameter("xown", [NOWNP, D], dt.float32, isOutput=False)
    waug_p = nc.declare_dram_parameter("waug", [D + 1, 5 * D], dt.float16, isOutput=False)
    wa_p = nc.declare_dram_parameter("wa", [D, D], dt.float32, isOutput=False)
    gb_p = nc.declare_dram_parameter("gb", [2, D], dt.float32, isOutput=False)
    srcidx_p = nc.declare_dram_parameter("srcidx", [GROUP, NT], dt.int32, isOutput=False)
    dstidx_p = nc.declare_dram_parameter("dstidx", [GROUP, NT], dt.int32, isOutput=False)
    out_p = nc.declare_dram_parameter("out", [NOWNP, D], dt.float32, isOutput=True)
    ktmt = [nc.dram_tensor(f"ktmt{s}", [NPAD, 2 * D], dt.float16) for s in (0, 1)]
    q_d = nc.dram_tensor("q", [NOWNP, D], dt.float16)
    acc_d = nc.dram_tensor("acc", [NOWNP, 72], dt.float32)

    PCH = NPAD // 2048  # projection chunks of 2048 nodes
    QCH = NOWNP // 128  # chunks holding q rows

    with tile.TileContext(nc) as tc:
        import contextlib
        with contextlib.ExitStack() as ctx:
            singles = ctx.enter_context(tc.tile_pool(name="singles", bufs=1))
            waug_t = singles.tile([D + 1, 5 * D], dt.float16)
            nc.sync.dma_start(out=waug_t[:], in_=waug_p[:])
            # zero the accumulator
            z = singles.tile([128, QCH, 72], dt.float32)
            nc.vector.memset(z[:], 0.0)
            nc.sync.dma_start(out=acc_d[:].rearrange("(a b) e -> b a e", b=128), in_=z[:])

            # ---- P0: projections ----
            with tc.tile_pool(name="pxt", bufs=2) as pxt, \
                 tc.tile_pool(name="pps", bufs=4, space="PSUM") as pps, \
                 tc.tile_pool(name="pev", bufs=2) as pev:
                for ch in range(PCH):
                    r0 = ch * 2048
                    xt = pxt.tile([D + 1, 2048], dt.float16)
                    nc.sync.dma_start_transpose(out=xt[:D, :], in_=x16_p[r0:r0 + 2048, :])
                    nc.vector.memset(xt[D:D + 1, :], 1.0)
                    km0 = pev.tile([128, 16, 2 * D], dt.float16, tag="km0")
                    km1 = pev.tile([128, 16, 2 * D], dt.float16, tag="km1")
                    qv = pev.tile([128, 16, D], dt.float16, tag="qv")
                    for j in range(16):
                        ps = pps.tile([128, 5 * D], dt.float32)
                        nc.tensor.matmul(out=ps[:], lhsT=xt[:, j * 128:(j + 1) * 128],
                                         rhs=waug_t[:], start=True, stop=True)
                        nc.vector.tensor_copy(out=km0[:, j, :], in_=ps[:, 0:128])
                        nc.vector.tensor_copy(out=km1[:, j, :], in_=ps[:, 128:256])
                        if ch * 16 + j < QCH:
                            nc.vector.tensor_copy(out=qv[:, j, :], in_=ps[:, 256:320])
                    for s, kmt in ((0, km0), (1, km1)):
                        nc.sync.dma_start(
                            out=ktmt[s][r0:r0 + 2048, :].rearrange("(a b) e -> b a e", b=128),
                            in_=kmt[:])
                    if ch * 16 < QCH:
                        hi = min(16, QCH - ch * 16)
                        nc.sync.dma_start(
                            out=q_d[r0:r0 + hi * 128, :].rearrange("(a b) e -> b a e", b=128),
                            in_=qv[:, :hi, :])

            # ---- P1: edge pipeline ----
            NT4 = NT // TILE_G
            with tc.tile_pool(name="eidx", bufs=2) as eidx, \
                 tc.tile_pool(name="egat", bufs=2) as egat, \
                 tc.tile_pool(name="epay", bufs=2) as epay, \
                 tc.tile_pool(name="esc", bufs=2) as esc:
                for t in range(NT4):
                    g0 = t * TILE_G
                    tab = ktmt[0] if g0 < ng[0] else ktmt[1]
                    sit = eidx.tile([128, TILE_G], dt.int32, tag="si")
                    nc.sync.dma_start(out=sit[:], in_=srcidx_p[:, g0:g0 + TILE_G])
                    dit = eidx.tile([128, TILE_G], dt.int32, tag="di")
                    nc.sync.dma_start(out=dit[:], in_=dstidx_p[:, g0:g0 + TILE_G])
                    kg = egat.tile([128, TILE_G, 2 * D], dt.float16, tag="kg")
                    qg = egat.tile([128, TILE_G, D], dt.float16, tag="qg")
                    for j in range(TILE_G):
                        nc.gpsimd.indirect_dma_start(
                            out=kg[:, j, :], out_offset=None, in_=tab[:],
                            in_offset=bass.IndirectOffsetOnAxis(ap=sit[:, j:j + 1], axis=0))
                        nc.gpsimd.indirect_dma_start(
                            out=qg[:, j, :], out_offset=None, in_=q_d[:],
                            in_offset=bass.IndirectOffsetOnAxis(ap=dit[:, j:j + 1], axis=0))
                    pr = esc.tile([128, TILE_G, D], dt.float32, tag="pr")
                    nc.vector.tensor_tensor(out=pr[:], in0=kg[:, :, 0:D], in1=qg[:],
                                            op=mybir.AluOpType.mult)
                    sco = esc.tile([128, TILE_G, H], dt.float32, tag="sco")
                    nc.vector.tensor_reduce(
                        out=sco[:], in_=pr[:].rearrange("p a (h c) -> p a h c", h=H),
                        axis=mybir.AxisListType.X, op=mybir.AluOpType.add)
                    nc.scalar.activation(out=sco[:], in_=sco[:],
                                         func=mybir.ActivationFunctionType.Exp)
                    stage = esc.tile([128, TILE_G, 72], dt.float32, tag="stage")
                    sap = sco[:]
                    wb = bass.AP(tensor=sap.tensor, offset=sap.offset,
                                 ap=[list(sap.ap[0]), list(sap.ap[1]),
                                     list(sap.ap[2]), [0, C]])
                    nc.vector.tensor_tensor(
                        out=stage[:, :, 0:D].rearrange("p a (h c) -> p a h c", h=H),
                        in0=kg[:, :, D:2 * D].rearrange("p a (h c) -> p a h c", h=H),
                        in1=wb, op=mybir.AluOpType.mult)
                    nc.vector.tensor_copy(out=stage[:, :, D:D + H], in_=sco[:])
                    for j in range(TILE_G):
                        pay = epay.tile([128, 72], dt.float32, tag="pay")
                        nc.vector.tensor_copy(out=pay[:], in_=stage[:, j, :])
                        nc.gpsimd.indirect_dma_start(
                            out=acc_d[:], out_offset=bass.IndirectOffsetOnAxis(
                                ap=dit[:, j:j + 1], axis=0),
                            in_=pay[:], in_offset=None,
                            compute_op=mybir.AluOpType.add)

            # ---- P2: finalize ----
            W2 = 2
            wa_t = singles.tile([D, D], dt.float32)
            nc.sync.dma_start(out=wa_t[:], in_=wa_p[:])
            gb_t = singles.tile([2, D], dt.float32)
            nc.sync.dma_start(out=gb_t[:], in_=gb_p[:])
            ident = singles.tile([128, 128], dt.float32)
            from concourse.masks import make_identity
            make_identity(nc, ident[:])
            eps_t = singles.tile([128, 1], dt.float32)
            nc.vector.memset(eps_t[:], EPS)
            with tc.tile_pool(name="f_in", bufs=2) as f_in, \
                 tc.tile_pool(name="f_ps", bufs=4, space="PSUM") as f_ps, \
                 tc.tile_pool(name="f_tmp", bufs=2) as f_tmp:
                for it in range(QCH // W2):
                    r0 = it * W2 * 128
                    at = f_in.tile([128, W2, 72], dt.float32, tag="at")
                    nc.sync.dma_start(
                        out=at[:], in_=acc_d[r0:r0 + W2 * 128, :].rearrange(
                            "(a b) e -> b a e", b=128))
                    xot = f_in.tile([128, W2, D], dt.float32, tag="xot")
                    nc.sync.dma_start(
                        out=xot[:], in_=xown_p[r0:r0 + W2 * 128, :].rearrange(
                            "(a b) e -> b a e", b=128))
                    den = f_tmp.tile([128, W2, H], dt.float32, tag="den")
                    # clamp denom==0 (isolated nodes / junk rows) to 1
                    iszero = f_tmp.tile([128, W2, H], dt.float32, tag="isz")
                    nc.vector.memset(iszero[:], 0.0)
                    nc.vector.tensor_tensor(out=iszero[:], in0=at[:, :, D:D + H],
                                            in1=iszero[:], op=mybir.AluOpType.is_equal)
                    nc.vector.tensor_tensor(out=den[:], in0=at[:, :, D:D + H],
                                            in1=iszero[:], op=mybir.AluOpType.add)
                    rec = f_tmp.tile([128, W2, H], dt.float32, tag="rec")
                    nc.vector.reciprocal(out=rec[:], in_=den[:])
                    rap = rec[:]
                    rb = bass.AP(tensor=rap.tensor, offset=rap.offset,
                                 ap=[list(rap.ap[0]), list(rap.ap[1]),
                                     list(rap.ap[2]), [0, C]])
                    g = f_tmp.tile([128, W2, D], dt.float32, tag="g")
                    nc.vector.tensor_tensor(
                        out=g[:].rearrange("p a (h c) -> p a h c", h=H),
                        in0=at[:, :, 0:D].rearrange("p a (h c) -> p a h c", h=H),
                        in1=rb, op=mybir.AluOpType.mult)
                    nc.scalar.activation(out=g[:], in_=g[:],
                                         func=mybir.ActivationFunctionType.Gelu)
                    y = f_tmp.tile([128, W2, D], dt.float32, tag="y")
                    for j in range(W2):
                        gt = f_ps.tile([64, 128], dt.float32, tag="gt")
                        nc.tensor.transpose(out=gt[:], in_=g[:, j, :], identity=ident[:])
                        gts = f_tmp.tile([64, 128], dt.float32, tag="gts")
                        nc.vector.tensor_copy(out=gts[:], in_=gt[:])
                        agg = f_ps.tile([128, D], dt.float32, tag="agg")
                        nc.tensor.matmul(out=agg[:], lhsT=gts[:], rhs=wa_t[:],
                                         start=True, stop=True)
                        nc.vector.tensor_scalar_mul(y[:, j, :], agg[:], sc)
                    ysk = f_tmp.tile([128, W2, D], dt.float32, tag="ysk")
                    nc.vector.tensor_scalar_mul(ysk[:], xot[:], 1.0 - sc)
                    nc.vector.tensor_tensor(out=y[:], in0=y[:], in1=ysk[:],
                                            op=mybir.AluOpType.add)
                    # layernorm over feature dim
                    st = f_tmp.tile([128, W2, 6], dt.float32, tag="st")
                    mv = f_tmp.tile([128, W2, 2], dt.float32, tag="mv")
                    for j in range(W2):
                        nc.vector.bn_stats(out=st[:, j, :], in_=y[:, j, :])
                        nc.vector.bn_aggr(out=mv[:, j, :], in_=st[:, j, :])
                    rstd = f_tmp.tile([128, W2], dt.float32, tag="rstd")
                    nc.scalar.activation(out=rstd[:], in_=mv[:, :, 1],
                                         func=mybir.ActivationFunctionType.Sqrt,
                                         bias=eps_t[:], scale=1.0)
                    nc.vector.reciprocal(out=rstd[:], in_=rstd[:])
                    mab = mv[:, :, 0:1]
                    mb = bass.AP(tensor=mab.tensor, offset=mab.offset,
                                 ap=[list(mab.ap[0]), list(mab.ap[1]), [0, D]])
                    nc.vector.tensor_tensor(out=y[:], in0=y[:], in1=mb,
                                            op=mybir.AluOpType.subtract)
                    rsap = rstd[:]
                    rsb = bass.AP(tensor=rsap.tensor, offset=rsap.offset,
                                  ap=[list(rsap.ap[0]), list(rsap.ap[1]), [0, D]])
                    nc.vector.tensor_tensor(out=y[:], in0=y[:], in1=rsb,
                                            op=mybir.AluOpType.mult)
                    if _APPLY_GB:
                        gap = gb_t[0:1, :]
                        gbc = bass.AP(tensor=gap.tensor, offset=gap.offset,
                                      ap=[[0, 128], [0, W2], list(gap.ap[1])])
                        nc.vector.tensor_tensor(out=y[:], in0=y[:], in1=gbc,
                                                op=mybir.AluOpType.mult)
                        bap = gb_t[1:2, :]
                        bbc = bass.AP(tensor=bap.tensor, offset=bap.offset,
                                      ap=[[0, 128], [0, W2], list(bap.ap[1])])
                        nc.vector.tensor_tensor(out=y[:], in0=y[:], in1=bbc,
                                                op=mybir.AluOpType.add)
                    nc.sync.dma_start(
                        out=out_p[r0:r0 + W2 * 128, :].rearrange("(a b) e -> b a e", b=128),
                        in_=y[:])

    _split_excess_waits(nc, 1)
    res = run_bass_kernel_spmd(nc, in_maps, list(range(NCORES)))
    outs = [res.results[c]["out"][:NOWN] for c in range(NCORES)]
    return np.concatenate(outs, axis=0).astype(np.float32), res


def _split_excess_waits(nc, max_waits=1):
    """walrus codegen rejects instructions with too many sem waits; hoist
    excess onto preceding same-engine NoOps."""
    import concourse.mybir as mybir
    n = 0
    for fn in nc.m.functions:
        for blk in fn.blocks:
            insts = blk.instructions
            new_list = []
            for inst in insts:
                si = inst.sync_info
                waits = list(si.on_wait) if si and si.on_wait else []
                if len(waits) > max_waits:
                    excess = waits[:-max_waits]
                    for j in range(0, len(excess), max_waits):
                        grp = excess[j:j + max_waits]
                        new_list.append(mybir.InstNoOp(
                            name=f"{inst.name}-ws{j}", engine=inst.engine,
                            ins=[], outs=[],
                            sync_info=mybir.SyncInfo(on_wait=grp, on_update=[]),
                            text_hint="wait_split", bass_nofuse=True))
                        n += 1
                    si.on_wait = waits[-max_waits:]
                new_list.append(inst)
            if len(new_list) != len(insts):
                insts[:] = new_list
    return n


_LAST_RESULT = {}


def kernel(**inputs):
    out, res = _build_and_run(inputs)
    _LAST_RESULT["res"] = res
    return out



# revision 3
# speedup vs baseline: 95.4859x; 95.4859x over previous
"""HGT graph update kernel for 8 Trainium2 NeuronCores.

Sharding: edge-parallel by destination-node range. Core c owns dst nodes
[c*12500, (c+1)*12500); its edges (from both edge sets) are routed to it by
the host. Node tensors are rotated per core so every core runs the same SPMD
program with its own range first. No collectives: per-core outputs are
disjoint row ranges, concatenated on the host.

Device pipeline per core:
  P0: project kt/mt (attention/message weights folded with Wk/Wm on host)
      for all N nodes and q for the own range, via TensorE from a
      DMA-transposed fp16 copy of x.
  P1: per 128-edge group: indirect-gather ktmt[src] and q[dst], score =
      sum_c kt*q per head, w = exp(score) (no max-subtraction needed: scores
      are O(0.1), softmax is shift-invariant), payload = [w*mt | w] and
      indirect scatter-add into a [12544, 72] accumulator. Host pre-groups
      edges into occurrence levels so no dst repeats within a group or in
      adjacent groups (scatter-add races otherwise).
  P2: pooled = numer/denom, gelu, @Wa, weighted skip, layernorm.
"""

import numpy as np

N = 100_000
D = 64
H, C = 8, 8
EPS = 1e-3
RSQRT_C = np.float32(1.0 / np.sqrt(C))
NCORES = 8
NOWN = 12500          # dst nodes per core
NOWNP = 12544         # padded (98*128); rows 12500+ are junk
NJUNK0 = 12500
NPAD = 100352         # 49*2048, x rows padded
GROUP = 128           # edges per indirect DMA
TILE_G = 32           # groups per edge tile (4096 edges)


def _block_diag(W):  # [H, C, C] -> [D, D]
    out = np.zeros((D, D), np.float32)
    for h in range(H):
        out[h * C:(h + 1) * C, h * C:(h + 1) * C] = W[h]
    return out


def _prep_core_edges(src, dst, base):
    """Edges with dst in [base, base+12500): returns (src_rot, dst_loc)
    grouped into occurrence levels; each level padded to GROUP multiple and
    followed by one junk group, so no dst repeats within a group or within
    adjacent groups."""
    sel = (dst >= base) & (dst < base + NOWN)
    s = ((src[sel].astype(np.int64) - base) % N).astype(np.int32)
    d = (dst[sel] - base).astype(np.int32)
    order = np.argsort(d, kind="stable")
    s, d = s[order], d[order]
    uniq, first, counts = np.unique(d, return_index=True, return_counts=True)
    occ = np.arange(d.size) - np.repeat(first, counts)
    lvl_order = np.argsort(occ, kind="stable")
    s, d, occ = s[lvl_order], d[lvl_order], occ[lvl_order]
    out_s, out_d = [], []
    junk = lambda n, k: (NJUNK0 + (np.arange(n) + k) % (NOWNP - NJUNK0)).astype(np.int32)
    for b in range(occ.max() + 1 if occ.size else 0):
        m = occ == b
        ls, ld = s[m], d[m]
        pad = (-ls.size) % GROUP
        out_s.append(ls); out_d.append(ld)
        out_s.append(np.zeros(pad + GROUP, np.int32))
        out_d.append(junk(pad + GROUP, b))
    return np.concatenate(out_s), np.concatenate(out_d)


def _prepare(inputs):
    """Host prep + bass build: returns (nc, in_maps)."""
    import concourse.bass as bass
    import concourse.tile as tile
    import concourse.mybir as mybir

    x = np.asarray(inputs["x"], np.float32)
    Wk, bk = np.asarray(inputs["Wk"]), np.asarray(inputs["bk"])
    Wm, bm = np.asarray(inputs["Wm"]), np.asarray(inputs["bm"])
    Wq, bq = np.asarray(inputs["Wq"]), np.asarray(inputs["bq"])
    Wa, ba = np.asarray(inputs["Wa"]), np.asarray(inputs["ba"])
    sc = float(1.0 / (1.0 + np.exp(-np.asarray(inputs["skip_w"])[0])))
    gamma, beta = np.asarray(inputs["ln_gamma"]), np.asarray(inputs["ln_beta"])

    # fold per-set head projections + prior*rsqrtC into the dense weights
    Wcols, bcols = [], []
    for s in (0, 1):
        BDa = _block_diag(np.asarray(inputs[f"Watt{s}"]))
        BDa *= np.repeat(np.asarray(inputs[f"prior{s}"]) * RSQRT_C, C)[None, :]
        BDm = _block_diag(np.asarray(inputs[f"Wmsg{s}"]))
        Wcols += [Wk @ BDa, Wm @ BDm]
        bcols += [bk @ BDa, bm @ BDm]
    Wcols.append(Wq); bcols.append(bq)
    Waug = np.concatenate([np.concatenate(Wcols, 1),
                           np.concatenate(bcols)[None, :]], 0).astype(np.float16)

    # per-core edge arrays
    per_core = []
    maxg = [0, 0]
    for c in range(NCORES):
        base = c * NOWN
        e = []
        for s in (0, 1):
            es, ed = _prep_core_edges(np.asarray(inputs[f"src{s}"]),
                                      np.asarray(inputs[f"dst{s}"]), base)
            e.append((es, ed))
            maxg[s] = max(maxg[s], es.size // GROUP)
        per_core.append(e)
    # pad each set's group count to TILE_G multiple, same on all cores
    ng = [-(-m // TILE_G) * TILE_G for m in maxg]
    NT = ng[0] + ng[1]
    in_maps = []
    for c in range(NCORES):
        base = c * NOWN
        si = np.zeros((NT * GROUP,), np.int32)
        di = np.tile(NJUNK0 + np.arange(GROUP) % (NOWNP - NJUNK0),
                     NT).astype(np.int32)
        off = 0
        for s in (0, 1):
            es, ed = per_core[c][s]
            si[off:off + es.size] = es
            di[off:off + ed.size] = ed
            off = ng[0] * GROUP
        xr = np.roll(x, -base, axis=0)
        x16 = np.zeros((NPAD, D), np.float16)
        x16[:N] = xr.astype(np.float16)
        in_maps.append({
            "x16": x16,
            "xown": np.ascontiguousarray(xr[:NOWNP]),
            "waug": Waug,
            "wa": np.ascontiguousarray(Wa.astype(np.float32)),
            "gb": np.stack([gamma, beta]).astype(np.float32),
            "srcidx": np.ascontiguousarray(si.reshape(NT, GROUP).T),
            "dstidx": np.ascontiguousarray(di.reshape(NT, GROUP).T),
        })

    _APPLY_GB = not (np.allclose(gamma, 1.0) and np.allclose(beta, 0.0))
    nc = bass.Bass()
    dt = mybir.dt
    x16_p = nc.declare_dram_parameter("x16", [NPAD, D], dt.float16, isOutput=False)
    xown_p = nc.declare_dram_parameter("xown", [NOWNP, D], dt.float32, isOutput=False)
    waug_p = nc.declare_dram_parameter("waug", [D + 1, 5 * D], dt.float16, isOutput=False)
    wa_p = nc.declare_dram_parameter("wa", [D, D], dt.float32, isOutput=False)
    gb_p = nc.declare_dram_parameter("gb", [2, D], dt.float32, isOutput=False)
    srcidx_p = nc.declare_dram_parameter("srcidx", [GROUP, NT], dt.int32, isOutput=False)
    dstidx_p = nc.declare_dram_parameter("dstidx", [GROUP, NT], dt.int32, isOutput=False)
    out_p = nc.declare_dram_parameter("out", [NOWNP, D], dt.float32, isOutput=True)
    ktmt = [nc.dram_tensor(f"ktmt{s}", [NPAD, 2 * D], dt.float16) for s in (0, 1)]
    q_d = nc.dram_tensor("q", [NOWNP, D], dt.float16)
    acc_d = nc.dram_tensor("acc", [NOWNP, 72], dt.float32)

    PCH = NPAD // 2048  # projection chunks of 2048 nodes
    QCH = NOWNP // 128  # chunks holding q rows

    with tile.TileContext(nc) as tc:
        import contextlib
        with contextlib.ExitStack() as ctx:
            singles = ctx.enter_context(tc.tile_pool(name="singles", bufs=1))
            waug_t = singles.tile([D + 1, 5 * D], dt.float16)
            nc.sync.dma_start(out=waug_t[:], in_=waug_p[:])
            # zero the accumulator
            z = singles.tile([128, QCH, 72], dt.float32)
            nc.vector.memset(z[:], 0.0)
            nc.sync.dma_start(out=acc_d[:].rearrange("(a b) e -> b a e", b=128), in_=z[:])

            # ---- P0: projections ----
            with tc.tile_pool(name="pxt", bufs=2) as pxt, \
                 tc.tile_pool(name="pps", bufs=4, space="PSUM") as pps, \
                 tc.tile_pool(name="pev", bufs=2) as pev:
                for ch in range(PCH):
                    r0 = ch * 2048
                    xt = pxt.tile([D + 1, 2048], dt.float16)
                    nc.sync.dma_start_transpose(out=xt[:D, :], in_=x16_p[r0:r0 + 2048, :])
                    nc.vector.memset(xt[D:D + 1, :], 1.0)
                    km0 = pev.tile([128, 16, 2 * D], dt.float16, tag="km0")
                    km1 = pev.tile([128, 16, 2 * D], dt.float16, tag="km1")
                    qv = pev.tile([128, 16, D], dt.float16, tag="qv")
                    for j in range(16):
                        ps = pps.tile([128, 5 * D], dt.float32)
                        nc.tensor.matmul(out=ps[:], lhsT=xt[:, j * 128:(j + 1) * 128],
                                         rhs=waug_t[:], start=True, stop=True)
                        nc.vector.tensor_copy(out=km0[:, j, :], in_=ps[:, 0:128])
                        nc.vector.tensor_copy(out=km1[:, j, :], in_=ps[:, 128:256])
                        if ch * 16 + j < QCH:
                            nc.vector.tensor_copy(out=qv[:, j, :], in_=ps[:, 256:320])
                    for s, kmt in ((0, km0), (1, km1)):
                        nc.sync.dma_start(
                            out=ktmt[s][r0:r0 + 2048, :].rearrange("(a b) e -> b a e", b=128),
                            in_=kmt[:])
                    if ch * 16 < QCH:
                        hi = min(16, QCH - ch * 16)
                        nc.sync.dma_start(
                            out=q_d[r0:r0 + hi * 128, :].rearrange("(a b) e -> b a e", b=128),
                            in_=qv[:, :hi, :])

            # ---- P1: edge pipeline ----
            NT4 = NT // TILE_G
            with tc.tile_pool(name="eidx", bufs=2) as eidx, \
                 tc.tile_pool(name="egat", bufs=2) as egat, \
                 tc.tile_pool(name="epay", bufs=2) as epay, \
                 tc.tile_pool(name="esc", bufs=2) as esc:
                for t in range(NT4):
                    g0 = t * TILE_G
                    tab = ktmt[0] if g0 < ng[0] else ktmt[1]
                    sit = eidx.tile([128, TILE_G], dt.int32, tag="si")
                    nc.sync.dma_start(out=sit[:], in_=srcidx_p[:, g0:g0 + TILE_G])
                    dit = eidx.tile([128, TILE_G], dt.int32, tag="di")
                    nc.sync.dma_start(out=dit[:], in_=dstidx_p[:, g0:g0 + TILE_G])
                    kg = egat.tile([128, TILE_G, 2 * D], dt.float16, tag="kg")
                    qg = egat.tile([128, TILE_G, D], dt.float16, tag="qg")
                    for j in range(TILE_G):
                        nc.gpsimd.indirect_dma_start(
                            out=kg[:, j, :], out_offset=None, in_=tab[:],
                            in_offset=bass.IndirectOffsetOnAxis(ap=sit[:, j:j + 1], axis=0))
                        nc.gpsimd.indirect_dma_start(
                            out=qg[:, j, :], out_offset=None, in_=q_d[:],
                            in_offset=bass.IndirectOffsetOnAxis(ap=dit[:, j:j + 1], axis=0))
                    pr = esc.tile([128, TILE_G, D], dt.float32, tag="pr")
                    nc.vector.tensor_tensor(out=pr[:], in0=kg[:, :, 0:D], in1=qg[:],
                                            op=mybir.AluOpType.mult)
                    sco = esc.tile([128, TILE_G, H], dt.float32, tag="sco")
                    nc.vector.tensor_reduce(
                        out=sco[:], in_=pr[:].rearrange("p a (h c) -> p a h c", h=H),
                        axis=mybir.AxisListType.X, op=mybir.AluOpType.add)
                    nc.scalar.activation(out=sco[:], in_=sco[:],
                                         func=mybir.ActivationFunctionType.Exp)
                    stage = esc.tile([128, TILE_G, 72], dt.float32, tag="stage")
                    sap = sco[:]
                    wb = bass.AP(tensor=sap.tensor, offset=sap.offset,
                                 ap=[list(sap.ap[0]), list(sap.ap[1]),
                                     list(sap.ap[2]), [0, C]])
                    nc.vector.tensor_tensor(
                        out=stage[:, :, 0:D].rearrange("p a (h c) -> p a h c", h=H),
                        in0=kg[:, :, D:2 * D].rearrange("p a (h c) -> p a h c", h=H),
                        in1=wb, op=mybir.AluOpType.mult)
                    nc.vector.tensor_copy(out=stage[:, :, D:D + H], in_=sco[:])
                    for j in range(TILE_G):
                        pay = epay.tile([128, 72], dt.float32, tag="pay")
                        nc.vector.tensor_copy(out=pay[:], in_=stage[:, j, :])
                        nc.gpsimd.indirect_dma_start(
                            out=acc_d[:], out_offset=bass.IndirectOffsetOnAxis(
                                ap=dit[:, j:j + 1], axis=0),
                            in_=pay[:], in_offset=None,
                            compute_op=mybir.AluOpType.add)

            # ---- P2: finalize ----
            W2 = 2
            wa_t = singles.tile([D, D], dt.float32)
            nc.sync.dma_start(out=wa_t[:], in_=wa_p[:])
            gb_t = singles.tile([2, D], dt.float32)
            nc.sync.dma_start(out=gb_t[:], in_=gb_p[:])
            ident = singles.tile([128, 128], dt.float32)
            from concourse.masks import make_identity
            make_identity(nc, ident[:])
            eps_t = singles.tile([128, 1], dt.float32)
            nc.vector.memset(eps_t[:], EPS)
            with tc.tile_pool(name="f_in", bufs=2) as f_in, \
                 tc.tile_pool(name="f_ps", bufs=4, space="PSUM") as f_ps, \
                 tc.tile_pool(name="f_tmp", bufs=2) as f_tmp:
                for it in range(QCH // W2):
                    r0 = it * W2 * 128
                    at = f_in.tile([128, W2, 72], dt.float32, tag="at")
                    nc.sync.dma_start(
                        out=at[:], in_=acc_d[r0:r0 + W2 * 128, :].rearrange(
                            "(a b) e -> b a e", b=128))
                    xot = f_in.tile([128, W2, D], dt.float32, tag="xot")
                    nc.sync.dma_start(
                        out=xot[:], in_=xown_p[r0:r0 + W2 * 128, :].rearrange(
                            "(a b) e -> b a e", b=128))
                    den = f_tmp.tile([128, W2, H], dt.float32, tag="den")
                    # clamp denom==0 (isolated nodes / junk rows) to 1
                    iszero = f_tmp.tile([128, W2, H], dt.float32, tag="isz")
                    nc.vector.memset(iszero[:], 0.0)
                    nc.vector.tensor_tensor(out=iszero[:], in0=at[:, :, D:D + H],
                                            in1=iszero[:], op=mybir.AluOpType.is_equal)
                    nc.vector.tensor_tensor(out=den[:], in0=at[:, :, D:D + H],
                                            in1=iszero[:], op=mybir.AluOpType.add)
                    rec = f_tmp.tile([128, W2, H], dt.float32, tag="rec")
                    nc.vector.reciprocal(out=rec[:], in_=den[:])
                    rap = rec[:]
                    rb = bass.AP(tensor=rap.tensor, offset=rap.offset,
                                 ap=[list(rap.ap[0]), list(rap.ap[1]),
                                     list(rap.ap[2]), [0, C]])
                    g = f_tmp.tile([128, W2, D], dt.float32, tag="g")
                    nc.vector.tensor_tensor(
                        out=g[:].rearrange("p a (h c) -> p a h c", h=H),
                        in0=at[:, :, 0:D].rearrange("p a (h c) -> p a h c", h=H),
                        in1=rb, op=mybir.AluOpType.mult)
                    nc.scalar.activation(out=g[:], in_=g[:],
                                         func=mybir.ActivationFunctionType.Gelu)
                    y = f_tmp.tile([128, W2, D], dt.float32, tag="y")
                    for j in range(W2):
                        gt = f_ps.tile([64, 128], dt.float32, tag="gt")
                        nc.tensor.transpose(out=gt[:], in_=g[:, j, :], identity=ident[:])
                        gts = f_tmp.tile([64, 128], dt.float32, tag="gts")
                        nc.vector.tensor_copy(out=gts[:], in_=gt[:])
                        agg = f_ps.tile([128, D], dt.float32, tag="agg")
                        nc.tensor.matmul(out=agg[:], lhsT=gts[:], rhs=wa_t[:],
                                         start=True, stop=True)
                        nc.vector.tensor_scalar_mul(y[:, j, :], agg[:], sc)
                    ysk = f_tmp.tile([128, W2, D], dt.float32, tag="ysk")
                    nc.vector.tensor_scalar_mul(ysk[:], xot[:], 1.0 - sc)
                    nc.vector.tensor_tensor(out=y[:], in0=y[:], in1=ysk[:],
                                            op=mybir.AluOpType.add)
                    # layernorm over feature dim
                    st = f_tmp.tile([128, W2, 6], dt.float32, tag="st")
                    mv = f_tmp.tile([128, W2, 2], dt.float32, tag="mv")
                    for j in range(W2):
                        nc.vector.bn_stats(out=st[:, j, :], in_=y[:, j, :])
                        nc.vector.bn_aggr(out=mv[:, j, :], in_=st[:, j, :])
                    rstd = f_tmp.tile([128, W2], dt.float32, tag="rstd")
                    nc.scalar.activation(out=rstd[:], in_=mv[:, :, 1],
                                         func=mybir.ActivationFunctionType.Sqrt,
                                         bias=eps_t[:], scale=1.0)
                    nc.vector.reciprocal(out=rstd[:], in_=rstd[:])
                    mab = mv[:, :, 0:1]
                    mb = bass.AP(tensor=mab.tensor, offset=mab.offset,
                                 ap=[list(mab.ap[0]), list(mab.ap[1]), [0, D]])
                    nc.vector.tensor_tensor(out=y[:], in0=y[:], in1=mb,
                                            op=mybir.AluOpType.subtract)
                    rsap = rstd[:]
                    rsb = bass.AP(tensor=rsap.tensor, offset=rsap.offset,
                                  ap=[list(rsap.ap[0]), list(rsap.ap[1]), [0, D]])
                    nc.vector.tensor_tensor(out=y[:], in0=y[:], in1=rsb,
                                            op=mybir.AluOpType.mult)
                    if _APPLY_GB:
                        gap = gb_t[0:1, :]
                        gbc = bass.AP(tensor=gap.tensor, offset=gap.offset,
                                      ap=[[0, 128], [0, W2], list(gap.ap[1])])
                        nc.vector.tensor_tensor(out=y[:], in0=y[:], in1=gbc,
                                                op=mybir.AluOpType.mult)
                        bap = gb_t[1:2, :]
                        bbc = bass.AP(tensor=bap.tensor, offset=bap.offset,
                                      ap=[[0, 128], [0, W2], list(bap.ap[1])])
                        nc.vector.tensor_tensor(out=y[:], in0=y[:], in1=bbc,
                                                op=mybir.AluOpType.add)
                    nc.sync.dma_start(
                        out=out_p[r0:r0 + W2 * 128, :].rearrange("(a b) e -> b a e", b=128),
                        in_=y[:])

    _split_excess_waits(nc, 1)
    return nc, in_maps


def _build_and_run(inputs):
    from concourse.bass_utils import run_bass_kernel_spmd
    nc, in_maps = _prepare(inputs)
    res = run_bass_kernel_spmd(nc, in_maps, list(range(NCORES)))
    outs = [res.results[c]["out"][:NOWN] for c in range(NCORES)]
    return np.concatenate(outs, axis=0).astype(np.float32), res


def _split_excess_waits(nc, max_waits=1):
    """walrus codegen rejects instructions with too many sem waits; hoist
    excess onto preceding same-engine NoOps."""
    import concourse.mybir as mybir
    n = 0
    for fn in nc.m.functions:
        for blk in fn.blocks:
            insts = blk.instructions
            new_list = []
            for inst in insts:
                si = inst.sync_info
                waits = list(si.on_wait) if si and si.on_wait else []
                if len(waits) > max_waits:
                    excess = waits[:-max_waits]
                    for j in range(0, len(excess), max_waits):
                        grp = excess[j:j + max_waits]
                        new_list.append(mybir.InstNoOp(
                            name=f"{inst.name}-ws{j}", engine=inst.engine,
                            ins=[], outs=[],
                            sync_info=mybir.SyncInfo(on_wait=grp, on_update=[]),
                            text_hint="wait_split", bass_nofuse=True))
                        n += 1
                    si.on_wait = waits[-max_waits:]
                new_list.append(inst)
            if len(new_list) != len(insts):
                insts[:] = new_list
    return n


_LAST_RESULT = {}


def kernel(**inputs):
    out, res = _build_and_run(inputs)
    _LAST_RESULT["res"] = res
    return out

